# revision 13
# baseline (speedup 1.0000x reference)
"""BayesLinearMF Trainium2 kernel.

Computes, for inputs of the reference nn_BayesLinearMF problem:
    weight = eps_w * exp(weight_psi)[None] + weight_mu[None]      (bs, out, in)
    out    = einsum("boi,bi->bo", weight, input)                  (bs, out)
    bias   = eps_b * exp(bias_psi)[None] + bias_mu[None]          (bs, out)
    return out + bias

Sharding: batch (bs=128) split across 8 NeuronCores (16 samples each);
weights replicated.  Per core the dominant cost is streaming its 64MB
eps_w shard from HBM (memory-bound, ~178us at ~360GB/s).

Per-core algorithm (o on partitions, i on the free axis):
  y[b, o] = sum_i eps[b,o,i] * S[o,i] * x[b,i]  + (x @ mu.T)[b,o] + bias[b,o]
with S = exp(psi).  eps arrives as bf16 via SWDGE cast-DMA (HBM reads
stay fp32 -> roofline unchanged, but DVE ops run in 2x bf16 mode).
Per b:
  pass1: slab *= XB_b       (merged over all 8 o-tiles, DVE 2x, in place;
                             XB_b = x[b,:] DMA-broadcast to 128 partitions)
  pass2: slab *= S          (merged, DVE 2x, in place)
  reduce: per o-tile, ScalarE activation(Copy) with accum_out -> the
          per-partition sums land directly in OUT_T[:, oc, b].
The mu-matvec + bias terms are computed once in (o, b) layout (PE
transposes + matmuls, scheduled into the DMA-bound tail) and added to
OUT_T at the end.  Output is stored (out, bs_local) per core; the host
transposes/concats.
"""

import os
import sys

import numpy as np

if os.path.isdir("/opt/trn_rl_repo") and "/opt/trn_rl_repo" not in sys.path:
    sys.path.insert(0, "/opt/trn_rl_repo")

from contextlib import ExitStack

import concourse.bacc as bacc
import concourse.bass as bass  # noqa: F401
import concourse.mybir as mybir
import concourse.tile as tile
from concourse.bass_utils import run_bass_kernel_spmd
from concourse.masks import make_identity

F32 = mybir.dt.float32
BF16 = mybir.dt.bfloat16
N_CORES = 8
BS = 128
BS_L = BS // N_CORES  # 16 per-core batch
D = 1024  # IN == OUT == 1024
P = 128
OC = D // P  # 8 o-tiles (and i-chunks)


def build_bass():
    nc = bacc.Bacc("TRN2")
    x = nc.dram_tensor("x", [BS_L, D], F32, kind="ExternalInput")
    mu = nc.dram_tensor("mu", [D, D], F32, kind="ExternalInput")
    psi = nc.dram_tensor("psi", [D, D], F32, kind="ExternalInput")
    bmu = nc.dram_tensor("bmu", [D], F32, kind="ExternalInput")
    bpsi = nc.dram_tensor("bpsi", [D], F32, kind="ExternalInput")
    eb = nc.dram_tensor("eb", [BS_L, D], F32, kind="ExternalInput")
    ew = nc.dram_tensor("ew", [BS_L, D, D], F32, kind="ExternalInput")
    out_t = nc.dram_tensor("out_t", [D, BS_L], F32, kind="ExternalOutput")

    Exp = mybir.ActivationFunctionType.Exp
    Ident = mybir.ActivationFunctionType.Identity
    Copy = mybir.ActivationFunctionType.Copy
    mult = mybir.AluOpType.mult

    with tile.TileContext(nc) as tc, ExitStack() as ctx:
        const = ctx.enter_context(tc.tile_pool(name="const", bufs=1))
        psum_tp = ctx.enter_context(
            tc.tile_pool(name="psum_tp", bufs=3, space="PSUM")
        )
        p2_psum = ctx.enter_context(
            tc.tile_pool(name="p2_psum", bufs=2, space="PSUM")
        )

        identity = const.tile([P, P], F32)
        make_identity(nc, identity)

        # S = exp(psi) in bf16: S_sb[p, t, i] = exp(psi[t*128+p, i])
        S_sb = const.tile([P, OC, D], BF16)
        with tc.tile_pool(name="psiload", bufs=1) as pp:
            psi_sb = pp.tile([P, OC, D], F32)
            nc.sync.dma_start(psi_sb, psi.rearrange("(t p) i -> p t i", p=P))
            nc.scalar.activation(S_sb, psi_sb, Exp)

        INIT = const.tile([P, OC, BS_L], F32)
        OUT_T = const.tile([P, OC, BS_L], F32)

        # ---------------- main loop (the memory-bound stream) ----------
        slab_pool = ctx.enter_context(tc.tile_pool(name="slab", bufs=4))
        xb_pool = ctx.enter_context(tc.tile_pool(name="xb", bufs=2))

        for b in range(BS_L):
            slab = slab_pool.tile([P, OC, D], BF16)
            nc.gpsimd.dma_start(slab, ew[b].rearrange("(t p) i -> p t i", p=P))

            XB = xb_pool.tile([P, D], BF16)
            nc.gpsimd.dma_start(XB, x[b : b + 1, :].to_broadcast((P, D)))

            nc.vector.tensor_tensor(
                slab[:], slab[:], XB[:, None, :].to_broadcast((P, OC, D)), mult
            )
            nc.vector.tensor_tensor(slab[:], slab[:], S_sb[:], mult)
            for oc in range(OC):
                P2 = p2_psum.tile([P, D], F32, tag="p2")
                nc.scalar.activation(
                    P2,
                    slab[:, oc, :],
                    Copy,
                    accum_out=OUT_T[:, oc, b : b + 1],
                )

        # ---------------- tail: INIT = (x @ mu.T).T + bias -------------
        # Scheduled into engine gaps / the DMA-bound tail (PE is idle in
        # the main loop; these ops only depend on mu/x/eb loads).
        with tc.tile_pool(name="setup", bufs=1) as setup:
            # mu.T : muT[p, ic, o] = mu[o, ic*128+p]
            mu_tmp = setup.tile([P, OC, D], F32)
            nc.sync.dma_start(mu_tmp, mu.rearrange("(t p) i -> p t i", p=P))
            muT = setup.tile([P, OC, D], F32)
            for oc in range(OC):
                for ic in range(OC):
                    pt = psum_tp.tile([P, P], F32, tag="ps")
                    nc.tensor.transpose(
                        pt, mu_tmp[:, oc, ic * P : (ic + 1) * P], identity
                    )
                    nc.vector.tensor_copy(
                        out=muT[:, ic, oc * P : (oc + 1) * P], in_=pt
                    )

            # x.T : xT[p, ic, b] = x[b, ic*128+p]
            x_sb = setup.tile([BS_L, D], F32)
            nc.sync.dma_start(x_sb, x[:])
            xT = setup.tile([P, OC, BS_L], F32)
            for ic in range(OC):
                pt = psum_tp.tile([P, BS_L], F32, tag="ps")
                nc.tensor.transpose(
                    pt, x_sb[:, ic * P : (ic + 1) * P], identity[:BS_L, :BS_L]
                )
                nc.vector.tensor_copy(out=xT[:, ic, :], in_=pt)

            # out2 = x @ mu.T  (b on partitions)
            out2_sb = setup.tile([BS_L, D], F32)
            for half in range(2):
                po = psum_tp.tile([BS_L, 512], F32, tag="ps")
                for ic in range(OC):
                    nc.tensor.matmul(
                        po,
                        lhsT=xT[:, ic, :],
                        rhs=muT[:, ic, half * 512 : (half + 1) * 512],
                        start=(ic == 0),
                        stop=(ic == OC - 1),
                    )
                nc.vector.tensor_copy(
                    out=out2_sb[:, half * 512 : (half + 1) * 512], in_=po
                )

            # eps_b.T : ebT[p, oc, b] = eps_b[b, oc*128+p]
            eb_sb = setup.tile([BS_L, D], F32)
            nc.sync.dma_start(eb_sb, eb[:])
            ebT = setup.tile([P, OC, BS_L], F32)
            for oc in range(OC):
                pt = psum_tp.tile([P, BS_L], F32, tag="ps")
                nc.tensor.transpose(
                    pt, eb_sb[:, oc * P : (oc + 1) * P], identity[:BS_L, :BS_L]
                )
                nc.vector.tensor_copy(out=ebT[:, oc, :], in_=pt)

            # per-partition bias columns: [p, oc] = v[oc*128+p]
            ebc = setup.tile([P, OC], F32)
            nc.sync.dma_start(ebc, bpsi.rearrange("(t p) -> p t", p=P))
            nc.scalar.activation(ebc, ebc, Exp)
            bmuc = setup.tile([P, OC], F32)
            nc.sync.dma_start(bmuc, bmu.rearrange("(t p) -> p t", p=P))

            # INIT = eps_b.T * exp(bias_psi) + bias_mu, then += out2.T
            for oc in range(OC):
                nc.scalar.activation(
                    INIT[:, oc, :],
                    ebT[:, oc, :],
                    Ident,
                    bias=bmuc[:, oc : oc + 1],
                    scale=ebc[:, oc : oc + 1],
                )
                pt = psum_tp.tile([P, BS_L], F32, tag="ps")
                nc.tensor.transpose(
                    pt,
                    out2_sb[:, oc * P : (oc + 1) * P],
                    identity[:BS_L, :BS_L],
                )
                nc.vector.tensor_add(
                    out=INIT[:, oc, :], in0=INIT[:, oc, :], in1=pt
                )

        nc.vector.tensor_add(out=OUT_T, in0=OUT_T, in1=INIT)
        nc.sync.dma_start(out_t.rearrange("(t p) b -> p t b", p=P), OUT_T)

    return nc


_NC_CACHE = None


def _get_nc():
    global _NC_CACHE
    if _NC_CACHE is None:
        _NC_CACHE = build_bass()
        if not _NC_CACHE.is_finalized():
            _NC_CACHE.finalize()
    return _NC_CACHE


def kernel(input, weight_mu, weight_psi, bias_mu, bias_psi, eps_w, eps_b,
           _trace=False):
    input = np.ascontiguousarray(np.asarray(input, dtype=np.float32))
    weight_mu = np.ascontiguousarray(np.asarray(weight_mu, dtype=np.float32))
    weight_psi = np.ascontiguousarray(np.asarray(weight_psi, dtype=np.float32))
    bias_mu = np.ascontiguousarray(np.asarray(bias_mu, dtype=np.float32))
    bias_psi = np.ascontiguousarray(np.asarray(bias_psi, dtype=np.float32))
    eps_w = np.ascontiguousarray(np.asarray(eps_w, dtype=np.float32))
    eps_b = np.ascontiguousarray(np.asarray(eps_b, dtype=np.float32))

    nc = _get_nc()
    in_maps = []
    for c in range(N_CORES):
        sl = slice(c * BS_L, (c + 1) * BS_L)
        in_maps.append(
            {
                "x": input[sl],
                "mu": weight_mu,
                "psi": weight_psi,
                "bmu": bias_mu,
                "bpsi": bias_psi,
                "eb": eps_b[sl],
                "ew": eps_w[sl],
            }
        )

    res = run_bass_kernel_spmd(
        nc, in_maps, core_ids=list(range(N_CORES)), trace=_trace
    )
    out = np.empty((BS, D), dtype=np.float32)
    for c in range(N_CORES):
        out[c * BS_L : (c + 1) * BS_L] = res.results[c]["out_t"].T
    if _trace:
        kernel.last_results = res
    return out


if __name__ == "__main__":
    rng = np.random.default_rng(0)
    inputs = {
        "input": rng.standard_normal((BS, D), dtype=np.float32),
        "weight_mu": rng.standard_normal((D, D), dtype=np.float32) * 0.03,
        "weight_psi": rng.uniform(-6, -5, (D, D)).astype(np.float32),
        "bias_mu": rng.standard_normal((D,), dtype=np.float32) * 0.03,
        "bias_psi": rng.uniform(-6, -5, (D,)).astype(np.float32),
        "eps_w": rng.standard_normal((BS, D, D), dtype=np.float32),
        "eps_b": rng.standard_normal((BS, D), dtype=np.float32),
    }
    out = kernel(**inputs)
    print(out.shape, out.dtype)


# revision 14
# speedup vs baseline: 1.2175x; 1.2175x over previous
"""BayesLinearMF Trainium2 kernel.

Computes, for inputs of the reference nn_BayesLinearMF problem:
    weight = eps_w * exp(weight_psi)[None] + weight_mu[None]      (bs, out, in)
    out    = einsum("boi,bi->bo", weight, input)                  (bs, out)
    bias   = eps_b * exp(bias_psi)[None] + bias_mu[None]          (bs, out)
    return out + bias

Sharding: batch (bs=128) split across 8 NeuronCores (16 samples each);
weights replicated.  Per core the dominant cost is streaming its 64MB
eps_w shard from HBM (memory-bound, ~178us at ~360GB/s).

Per-core algorithm (o on partitions, i on the free axis):
  y[b, o] = sum_i eps[b,o,i] * S[o,i] * x[b,i]  + (x @ mu.T)[b,o] + bias[b,o]
with S = exp(psi).  eps arrives as bf16 via SWDGE cast-DMA (HBM reads
stay fp32 -> roofline unchanged, but DVE ops run in 2x bf16 mode).
Per b:
  pass1: slab *= XB_b       (merged over all 8 o-tiles, DVE 2x, in place;
                             XB_b = x[b,:] DMA-broadcast to 128 partitions)
  pass2: slab *= S          (merged, DVE 2x, in place)
  reduce: per o-tile, ScalarE activation(Copy) with accum_out -> the
          per-partition sums land directly in OUT_T[:, oc, b].
The mu-matvec + bias terms are computed once in (o, b) layout (PE
transposes + matmuls, scheduled into the DMA-bound tail) and added to
OUT_T at the end.  Output is stored (out, bs_local) per core; the host
transposes/concats.
"""

import os
import sys

import numpy as np

if os.path.isdir("/opt/trn_rl_repo") and "/opt/trn_rl_repo" not in sys.path:
    sys.path.insert(0, "/opt/trn_rl_repo")

from contextlib import ExitStack

import concourse.bacc as bacc
import concourse.bass as bass  # noqa: F401
import concourse.mybir as mybir
import concourse.tile as tile
from concourse.bass_utils import run_bass_kernel_spmd
from concourse.masks import make_identity

F32 = mybir.dt.float32
BF16 = mybir.dt.bfloat16
N_CORES = 8
BS = 128
BS_L = BS // N_CORES  # 16 per-core batch
D = 1024  # IN == OUT == 1024
P = 128
OC = D // P  # 8 o-tiles (and i-chunks)


def build_bass():
    nc = bacc.Bacc("TRN2")
    x = nc.dram_tensor("x", [BS_L, D], F32, kind="ExternalInput")
    mu = nc.dram_tensor("mu", [D, D], F32, kind="ExternalInput")
    psi = nc.dram_tensor("psi", [D, D], F32, kind="ExternalInput")
    bmu = nc.dram_tensor("bmu", [D], F32, kind="ExternalInput")
    bpsi = nc.dram_tensor("bpsi", [D], F32, kind="ExternalInput")
    eb = nc.dram_tensor("eb", [BS_L, D], F32, kind="ExternalInput")
    ew = nc.dram_tensor("ew", [BS_L, D, D], F32, kind="ExternalInput")
    out_t = nc.dram_tensor("out_t", [D, BS_L], F32, kind="ExternalOutput")

    Exp = mybir.ActivationFunctionType.Exp
    Ident = mybir.ActivationFunctionType.Identity
    Copy = mybir.ActivationFunctionType.Copy
    mult = mybir.AluOpType.mult

    with tile.TileContext(nc) as tc, ExitStack() as ctx:
        const = ctx.enter_context(tc.tile_pool(name="const", bufs=1))
        psum_tp = ctx.enter_context(
            tc.tile_pool(name="psum_tp", bufs=3, space="PSUM")
        )
        p2_psum = ctx.enter_context(
            tc.tile_pool(name="p2_psum", bufs=2, space="PSUM")
        )

        identity = const.tile([P, P], F32)
        make_identity(nc, identity)

        # S = exp(psi) in bf16: S_sb[p, t, i] = exp(psi[t*128+p, i])
        S_sb = const.tile([P, OC, D], BF16)
        with tc.tile_pool(name="psiload", bufs=1) as pp:
            psi_sb = pp.tile([P, OC, D], F32)
            nc.sync.dma_start(psi_sb, psi.rearrange("(t p) i -> p t i", p=P))
            nc.scalar.activation(S_sb, psi_sb, Exp)

        INIT = const.tile([P, OC, BS_L], F32)
        OUT_T = const.tile([P, OC, BS_L], F32)

        # ---------------- main loop (the memory-bound stream) ----------
        slab_pool = ctx.enter_context(tc.tile_pool(name="slab", bufs=3))
        xb_pool = ctx.enter_context(tc.tile_pool(name="xb", bufs=2))

        for b in range(BS_L):
            slab = slab_pool.tile([P, OC, D], BF16)
            nc.gpsimd.dma_start(slab, ew[b].rearrange("(t p) i -> p t i", p=P))

            XB = xb_pool.tile([P, D], BF16)
            nc.gpsimd.dma_start(XB, x[b : b + 1, :].to_broadcast((P, D)))

            nc.vector.tensor_tensor(
                slab[:], slab[:], XB[:, None, :].to_broadcast((P, OC, D)), mult
            )
            nc.vector.tensor_tensor(slab[:], slab[:], S_sb[:], mult)
            for oc in range(OC):
                P2 = p2_psum.tile([P, D], F32, tag="p2")
                nc.scalar.activation(
                    P2,
                    slab[:, oc, :],
                    Copy,
                    accum_out=OUT_T[:, oc, b : b + 1],
                )

        # ---------------- tail: INIT = (x @ mu.T).T + bias -------------
        # Scheduled into engine gaps / the DMA-bound tail (PE is idle in
        # the main loop; these ops only depend on mu/x/eb loads).
        with tc.tile_pool(name="setup", bufs=1) as setup:
            # mu.T : muT[p, ic, o] = mu[o, ic*128+p]
            mu_tmp = setup.tile([P, OC, D], F32)
            nc.sync.dma_start(mu_tmp, mu.rearrange("(t p) i -> p t i", p=P))
            muT = setup.tile([P, OC, D], F32)
            for oc in range(OC):
                for ic in range(OC):
                    pt = psum_tp.tile([P, P], F32, tag="ps")
                    nc.tensor.transpose(
                        pt, mu_tmp[:, oc, ic * P : (ic + 1) * P], identity
                    )
                    nc.vector.tensor_copy(
                        out=muT[:, ic, oc * P : (oc + 1) * P], in_=pt
                    )

            # x.T : xT[p, ic, b] = x[b, ic*128+p]
            x_sb = setup.tile([BS_L, D], F32)
            nc.sync.dma_start(x_sb, x[:])
            xT = setup.tile([P, OC, BS_L], F32)
            for ic in range(OC):
                pt = psum_tp.tile([P, BS_L], F32, tag="ps")
                nc.tensor.transpose(
                    pt, x_sb[:, ic * P : (ic + 1) * P], identity[:BS_L, :BS_L]
                )
                nc.vector.tensor_copy(out=xT[:, ic, :], in_=pt)

            # out2 = x @ mu.T  (b on partitions)
            out2_sb = setup.tile([BS_L, D], F32)
            for half in range(2):
                po = psum_tp.tile([BS_L, 512], F32, tag="ps")
                for ic in range(OC):
                    nc.tensor.matmul(
                        po,
                        lhsT=xT[:, ic, :],
                        rhs=muT[:, ic, half * 512 : (half + 1) * 512],
                        start=(ic == 0),
                        stop=(ic == OC - 1),
                    )
                nc.vector.tensor_copy(
                    out=out2_sb[:, half * 512 : (half + 1) * 512], in_=po
                )

            # eps_b.T : ebT[p, oc, b] = eps_b[b, oc*128+p]
            eb_sb = setup.tile([BS_L, D], F32)
            nc.sync.dma_start(eb_sb, eb[:])
            ebT = setup.tile([P, OC, BS_L], F32)
            for oc in range(OC):
                pt = psum_tp.tile([P, BS_L], F32, tag="ps")
                nc.tensor.transpose(
                    pt, eb_sb[:, oc * P : (oc + 1) * P], identity[:BS_L, :BS_L]
                )
                nc.vector.tensor_copy(out=ebT[:, oc, :], in_=pt)

            # per-partition bias columns: [p, oc] = v[oc*128+p]
            ebc = setup.tile([P, OC], F32)
            nc.sync.dma_start(ebc, bpsi.rearrange("(t p) -> p t", p=P))
            nc.scalar.activation(ebc, ebc, Exp)
            bmuc = setup.tile([P, OC], F32)
            nc.sync.dma_start(bmuc, bmu.rearrange("(t p) -> p t", p=P))

            # INIT = eps_b.T * exp(bias_psi) + bias_mu, then += out2.T
            for oc in range(OC):
                nc.scalar.activation(
                    INIT[:, oc, :],
                    ebT[:, oc, :],
                    Ident,
                    bias=bmuc[:, oc : oc + 1],
                    scale=ebc[:, oc : oc + 1],
                )
                pt = psum_tp.tile([P, BS_L], F32, tag="ps")
                nc.tensor.transpose(
                    pt,
                    out2_sb[:, oc * P : (oc + 1) * P],
                    identity[:BS_L, :BS_L],
                )
                nc.vector.tensor_add(
                    out=INIT[:, oc, :], in0=INIT[:, oc, :], in1=pt
                )

        nc.vector.tensor_add(out=OUT_T, in0=OUT_T, in1=INIT)
        nc.sync.dma_start(out_t.rearrange("(t p) b -> p t b", p=P), OUT_T)

    return nc


_NC_CACHE = None


def _get_nc():
    global _NC_CACHE
    if _NC_CACHE is None:
        _NC_CACHE = build_bass()
        if not _NC_CACHE.is_finalized():
            _NC_CACHE.finalize()
    return _NC_CACHE


def kernel(input, weight_mu, weight_psi, bias_mu, bias_psi, eps_w, eps_b,
           _trace=False):
    input = np.ascontiguousarray(np.asarray(input, dtype=np.float32))
    weight_mu = np.ascontiguousarray(np.asarray(weight_mu, dtype=np.float32))
    weight_psi = np.ascontiguousarray(np.asarray(weight_psi, dtype=np.float32))
    bias_mu = np.ascontiguousarray(np.asarray(bias_mu, dtype=np.float32))
    bias_psi = np.ascontiguousarray(np.asarray(bias_psi, dtype=np.float32))
    eps_w = np.ascontiguousarray(np.asarray(eps_w, dtype=np.float32))
    eps_b = np.ascontiguousarray(np.asarray(eps_b, dtype=np.float32))

    nc = _get_nc()
    in_maps = []
    for c in range(N_CORES):
        sl = slice(c * BS_L, (c + 1) * BS_L)
        in_maps.append(
            {
                "x": input[sl],
                "mu": weight_mu,
                "psi": weight_psi,
                "bmu": bias_mu,
                "bpsi": bias_psi,
                "eb": eps_b[sl],
                "ew": eps_w[sl],
            }
        )

    res = run_bass_kernel_spmd(
        nc, in_maps, core_ids=list(range(N_CORES)), trace=_trace
    )
    out = np.empty((BS, D), dtype=np.float32)
    for c in range(N_CORES):
        out[c * BS_L : (c + 1) * BS_L] = res.results[c]["out_t"].T
    if _trace:
        kernel.last_results = res
    return out


if __name__ == "__main__":
    rng = np.random.default_rng(0)
    inputs = {
        "input": rng.standard_normal((BS, D), dtype=np.float32),
        "weight_mu": rng.standard_normal((D, D), dtype=np.float32) * 0.03,
        "weight_psi": rng.uniform(-6, -5, (D, D)).astype(np.float32),
        "bias_mu": rng.standard_normal((D,), dtype=np.float32) * 0.03,
        "bias_psi": rng.uniform(-6, -5, (D,)).astype(np.float32),
        "eps_w": rng.standard_normal((BS, D, D), dtype=np.float32),
        "eps_b": rng.standard_normal((BS, D), dtype=np.float32),
    }
    out = kernel(**inputs)
    print(out.shape, out.dtype)
